# revision 10
# baseline (speedup 1.0000x reference)
"""Causal GQA self-attention on 8 Trainium2 NeuronCores.

Problem: B=2, S=2048, HIDDEN=2048, 16 q-heads, 4 kv-heads, head_dim=128, fp32.

Sharding: core c = 4*b + g  (b = batch, g = head-group).  Each core owns batch
b and q-heads [4g, 4g+4) plus their shared kv-head g.  No collectives: each
core computes the partial O-projection  Wo[own rows].T @ attn_own  ->
[2048 hid, 2048 q] (bf16) and the host sums the 4 partials per batch.

Per-core pipeline (feature-major [d, s] layouts, all matmuls bf16):
  A. Projections per 512-col chunk: QT/KT/VT = W.T @ X.T accumulated over the
     16 128-row hid tiles; V obtained by PE-transposing VT (bias folded in).
  B. Attention per chunk/head: for each key-tile pair, S = KT_j.T @ QT (PSUM
     pair tile), P = exp(S/sqrt(d)) (one ACT op per pair), causal masks
     multiplied on the 4 diagonal tiles (DVE), then PV and the row-sum l
     accumulate on PE.  attnT = PV * (1/l broadcast).
  C. O-projection at the end with weight reuse over chunks:
     partial[128 hid-tile, q] += Wo_tile.T @ attnT, evac + DMA out per chunk.

P_FP8 variant: P is written fp8e4m3 (exp shifted by -C so max < 240) and the
row-sum uses a DoubleRow fp8 matmul over key-tile pairs (half the PE cycles of
the bf16 row-sum); PV keeps the bf16 V stationary with the fp8 moving P.
"""

import numpy as np
import ml_dtypes

HID = 2048
S = 2048
B = 2
NH = 16          # q heads total
D = 128          # head dim
G = 4            # head groups == cores per batch
HPG = NH // G    # q heads per group (4)
CH = 512         # seq chunk
NCH = S // CH    # 4 chunks
NKT = S // 128   # 16 key tiles / hid tiles
SCALE = 1.0 / float(np.sqrt(D))

P_FP8 = True     # P in fp8e4m3 + DoubleRow row-sum (else all-bf16)
EXP_C = 2.0      # global shift inside exp when P_FP8 (cancels in softmax)

F8NP = ml_dtypes.float8_e4m3
BFNP = ml_dtypes.bfloat16

_CACHED_NC = None
_CACHED_KEY = None
_last_in_maps = None


def _build_nc(mask_has_zeros=False):
    import concourse.mybir as mybir
    import concourse.tile as tile
    from concourse import bacc

    F32 = mybir.dt.float32
    BF = mybir.dt.bfloat16
    F8 = mybir.dt.float8e4
    PDT = F8 if P_FP8 else BF
    Copy = mybir.ActivationFunctionType.Copy
    Exp = mybir.ActivationFunctionType.Exp
    Ident = mybir.ActivationFunctionType.Identity
    DR = mybir.MatmulPerfMode.DoubleRow

    nc = bacc.Bacc("TRN2", target_bir_lowering=False, debug=False,
                   num_devices=8)

    xt_d = nc.declare_dram_parameter("xt", [128, NKT, S], BF, isOutput=False)
    wq_d = nc.declare_dram_parameter("wq", [128, NKT, HPG * D], BF, isOutput=False)
    wk_d = nc.declare_dram_parameter("wk", [128, NKT, D], BF, isOutput=False)
    wv_d = nc.declare_dram_parameter("wv", [128, NKT, D], BF, isOutput=False)
    wo_d = nc.declare_dram_parameter("wo", [128, HPG, NKT, D], BF, isOutput=False)
    bq_d = nc.declare_dram_parameter("bq", [D, HPG], F32, isOutput=False)
    bk_d = nc.declare_dram_parameter("bk", [D, 1], F32, isOutput=False)
    bv_d = nc.declare_dram_parameter("bv", [D, 1], F32, isOutput=False)
    bo_d = nc.declare_dram_parameter("bo", [D, NKT], F32, isOutput=False)
    kb_d = nc.declare_dram_parameter("kb", [128, NKT], F32, isOutput=False)
    mask_d = nc.declare_dram_parameter("masks", [128, HPG, CH], PDT, isOutput=False)
    id_d = nc.declare_dram_parameter("ident", [128, 128], BF, isOutput=False)
    ones_d = nc.declare_dram_parameter("ones", [128, 2, 16], PDT, isOutput=False)
    out_d = nc.declare_dram_parameter("out", [HID, S], BF, isOutput=True)

    with tile.TileContext(nc) as tc:
        with tc.tile_pool(name="persist", bufs=1) as persist:
            xt = persist.tile([128, NKT, S], BF)
            wq = persist.tile([128, NKT, HPG * D], BF)
            wk = persist.tile([128, NKT, D], BF)
            wv = persist.tile([128, NKT, D], BF)
            wo = persist.tile([128, HPG, NKT, D], BF)
            qt = persist.tile([128, HPG, S], BF)
            kt = persist.tile([128, S], BF)
            v16 = persist.tile([128, NKT, D], BF)
            at16 = persist.tile([128, HPG, S], BF)
            masks = persist.tile([128, HPG, CH], PDT)
            kb = persist.tile([128, NKT], F32)
            bq_sb = persist.tile([D, HPG], F32)
            bk_sb = persist.tile([D, 1], F32)
            bv_sb = persist.tile([D, 1], F32)
            bo_sb = persist.tile([D, NKT], F32)
            ident = persist.tile([128, 128], BF)
            ones_sb = persist.tile([128, 2, 16], PDT)

            # Phase A consumes (wq[t], wk[t], wv[t], xt[t, chunk]) in t-order
            # per chunk.  Weights ride the scalar HWDGE queue, xt the sync
            # HWDGE queue (chunk 0 first), small consts the gpsimd queue, so
            # the streams run in parallel and the PE never outruns the DMA.
            for t in range(NKT):
                nc.scalar.dma_start(out=wq[:, t, :], in_=wq_d.ap()[:, t, :])
                nc.scalar.dma_start(out=wk[:, t, :], in_=wk_d.ap()[:, t, :])
                nc.scalar.dma_start(out=wv[:, t, :], in_=wv_d.ap()[:, t, :])
                nc.sync.dma_start(out=xt[:, t, 0:CH],
                                  in_=xt_d.ap()[:, t, 0:CH])
            nc.gpsimd.dma_start(out=bq_sb, in_=bq_d.ap())
            nc.gpsimd.dma_start(out=bk_sb, in_=bk_d.ap())
            nc.gpsimd.dma_start(out=bv_sb, in_=bv_d.ap())
            nc.gpsimd.dma_start(out=bo_sb, in_=bo_d.ap())
            nc.gpsimd.dma_start(out=kb, in_=kb_d.ap())
            nc.gpsimd.dma_start(out=ident, in_=id_d.ap())
            nc.gpsimd.dma_start(out=ones_sb, in_=ones_d.ap())
            nc.gpsimd.dma_start(out=masks, in_=mask_d.ap())
            for c in range(1, NCH):
                sq = slice(c * CH, (c + 1) * CH)
                for t in range(NKT):
                    eng = nc.sync if (t % 2 == 0) else nc.gpsimd
                    eng.dma_start(out=xt[:, t, sq], in_=xt_d.ap()[:, t, sq])
            nc.scalar.dma_start(out=wo, in_=wo_d.ap())

            # ================= Phase A: projections =================
            with (
                tc.tile_pool(name="psA", bufs=1, space="PSUM") as psA,
                tc.tile_pool(name="vts", bufs=2) as vts,
            ):
                for c in range(NCH):
                    sq = slice(c * CH, (c + 1) * CH)
                    ps_q = psA.tile([128, HPG, CH], F32, name="ps_q")
                    ps_k = psA.tile([128, CH], F32, name="ps_k")
                    ps_vt = psA.tile([128, CH], F32, name="ps_vt")
                    for t in range(NKT):
                        st, sp = (t == 0), (t == NKT - 1)
                        for h in range(HPG):
                            nc.tensor.matmul(
                                ps_q[:, h, :],
                                lhsT=wq[:, t, h * D:(h + 1) * D],
                                rhs=xt[:, t, sq], start=st, stop=sp)
                        nc.tensor.matmul(ps_k, lhsT=wk[:, t, :],
                                         rhs=xt[:, t, sq], start=st, stop=sp)
                        nc.tensor.matmul(ps_vt, lhsT=wv[:, t, :],
                                         rhs=xt[:, t, sq], start=st, stop=sp)
                    for h in range(HPG):
                        nc.vector.tensor_scalar_add(qt[:, h, sq],
                                                    ps_q[:, h, :],
                                                    bq_sb[:, h:h + 1])
                    nc.vector.tensor_scalar_add(kt[:, sq], ps_k, bk_sb)
                    vt_t = vts.tile([128, CH], BF, name="vt_t")
                    nc.scalar.activation(vt_t, ps_vt, Ident, bias=bv_sb[:, 0:1])
                    for u in range(4):
                        ps_tp = psA.tile([128, 128], BF, name="ps_tp",
                                         bufs=2)
                        nc.tensor.transpose(
                            ps_tp, vt_t[:, u * 128:(u + 1) * 128], ident)
                        nc.scalar.activation(v16[:, 4 * c + u, :], ps_tp, Copy)

            # ================= Phase B: attention =================
            with (
                tc.tile_pool(name="psS", bufs=2, space="PSUM") as psS,
                tc.tile_pool(name="psPV", bufs=2, space="PSUM") as psPV,
                tc.tile_pool(name="psL", bufs=2, space="PSUM") as psL,
                tc.tile_pool(name="pp", bufs=3) as ppool,
                tc.tile_pool(name="rbp", bufs=2) as rbp,
            ):
                for c in range(NCH):
                    sq = slice(c * CH, (c + 1) * CH)
                    njt = 4 * c + 4
                    for h in range(HPG):
                        ps_pv = psPV.tile([128, CH], F32, name="ps_pv")
                        ps_l = psL.tile([16, CH], F32, name="ps_l")
                        for pi in range(njt // 2):
                            j0 = 2 * pi
                            last = (pi == njt // 2 - 1)
                            # the last pair (key tiles 4c+2, 4c+3) only
                            # reaches queries >= 256 within the chunk
                            off = 256 if last else 0
                            w = CH - off
                            cs = slice(off, CH)
                            qcs = slice(c * CH + off, (c + 1) * CH)
                            ps_s = psS.tile([128, 2, CH], F32, name="ps_s")
                            for sl in range(2):
                                j = j0 + sl
                                nc.tensor.matmul(
                                    ps_s[:, sl, cs],
                                    lhsT=kt[:, j * 128:(j + 1) * 128],
                                    rhs=qt[:, h, qcs], start=True, stop=True)
                            p16 = ppool.tile([128, 2, CH], PDT, name="p16")
                            if mask_has_zeros:
                                for sl in range(2):
                                    j = j0 + sl
                                    nc.scalar.activation(
                                        p16[:, sl, cs], ps_s[:, sl, cs], Exp,
                                        scale=SCALE, bias=kb[:, j:j + 1])
                            else:
                                # kb is constant -shift when the mask has no
                                # zeros, so one bias AP serves the whole pair
                                nc.scalar.activation(
                                    p16[:, :, cs], ps_s[:, :, cs], Exp,
                                    scale=SCALE, bias=kb[:, 0:1])
                            if pi == 2 * c:  # diagonal pair 0: r = 0, 1
                                nc.vector.tensor_mul(
                                    p16[:, 0, 0:128], p16[:, 0, 0:128],
                                    masks[:, 0, 0:128])
                                nc.vector.tensor_mul(
                                    p16[:, 1, 0:256], p16[:, 1, 0:256],
                                    masks[:, 1, 0:256])
                            if last:         # diagonal pair 1: r = 2, 3
                                nc.vector.tensor_mul(
                                    p16[:, 0, 256:384], p16[:, 0, 256:384],
                                    masks[:, 2, 256:384])
                                nc.vector.tensor_mul(
                                    p16[:, 1, 256:512], p16[:, 1, 256:512],
                                    masks[:, 3, 256:512])
                            for sl in range(2):
                                j = j0 + sl
                                nc.tensor.matmul(
                                    ps_pv[:, cs], lhsT=v16[:, j, :],
                                    rhs=p16[:, sl, cs],
                                    start=(j == 0), stop=(j == njt - 1))
                            if P_FP8:
                                nc.tensor.matmul(
                                    ps_l[:, cs], lhsT=ones_sb,
                                    rhs=p16[:, :, cs], perf_mode=DR,
                                    start=(pi == 0), stop=last)
                            else:
                                for sl in range(2):
                                    j = j0 + sl
                                    nc.tensor.matmul(
                                        ps_l[:, cs], lhsT=ones_sb[:, 0, :],
                                        rhs=p16[:, sl, cs],
                                        start=(j == 0), stop=(j == njt - 1))
                        rl = rbp.tile([1, CH], F32, name="rl")
                        nc.vector.reciprocal_approx_fast(rl, ps_l[0:1, :])
                        rb = rbp.tile([128, CH], F32, name="rb")
                        nc.gpsimd.partition_broadcast(rb, rl, channels=128)
                        nc.vector.tensor_mul(at16[:, h, sq], ps_pv, rb)

            # ================= Phase C: partial O-projection =================
            with (
                tc.tile_pool(name="psO", bufs=2, space="PSUM") as psO,
                tc.tile_pool(name="ost", bufs=6) as ost,
            ):
                for tcol in range(NKT):
                    ps_o = psO.tile([128, NCH, CH], F32, name="ps_o")
                    for ft in range(HPG):
                        for c in range(NCH):
                            nc.tensor.matmul(
                                ps_o[:, c, :], lhsT=wo[:, ft, tcol, :],
                                rhs=at16[:, ft, c * CH:(c + 1) * CH],
                                start=(ft == 0), stop=(ft == HPG - 1))
                    for c in range(NCH):
                        o_sb = ost.tile([128, CH], BF, name="o_sb")
                        nc.vector.tensor_scalar_add(o_sb, ps_o[:, c, :],
                                                    bo_sb[:, tcol:tcol + 1])
                        eng = nc.sync if (c % 2 == 0) else nc.scalar
                        eng.dma_start(
                            out=out_d[tcol * 128:(tcol + 1) * 128,
                                      c * CH:(c + 1) * CH],
                            in_=o_sb)

    nc.compile()
    return nc


def _host_consts():
    # tri masks for the diagonal key tiles: masks[p, r, col] = col >= 128r + p
    col = np.arange(CH)[None, None, :]
    r = np.arange(HPG)[None, :, None]
    p = np.arange(128)[:, None, None]
    pdt = F8NP if P_FP8 else BFNP
    masks = (col >= 128 * r + p).astype(pdt)
    ident = np.eye(128, dtype=BFNP)
    ones = np.ones((128, 2, 16), dtype=pdt)
    return masks, ident, ones


def _pack_ft(w, m):
    # [HID, m] -> [128, NKT, m]: row 128*t + p goes to [p, t, :]
    return np.ascontiguousarray(
        w.reshape(NKT, 128, m).transpose(1, 0, 2).astype(BFNP))


def kernel(hidden_states, attention_mask, Wq, bq, Wk, bk, Wv, bv, Wo, bo):
    from concourse.bass_utils import run_bass_kernel_spmd

    X = np.asarray(hidden_states, dtype=np.float32)
    am = np.asarray(attention_mask).astype(np.float32)  # [B, S] key mask
    Wq = np.asarray(Wq, np.float32)
    Wk = np.asarray(Wk, np.float32)
    Wv = np.asarray(Wv, np.float32)
    Wo = np.asarray(Wo, np.float32)
    mask_has_zeros = bool((am == 0.0).any())

    global _CACHED_NC, _CACHED_KEY
    if _CACHED_NC is None or _CACHED_KEY != mask_has_zeros:
        _CACHED_NC = _build_nc(mask_has_zeros)
        _CACHED_KEY = mask_has_zeros
    nc = _CACHED_NC

    masks, ident, ones = _host_consts()
    shift = EXP_C if P_FP8 else 0.0

    xt_b = []
    kb_b = []
    for b in range(B):
        xt_b.append(np.ascontiguousarray(
            X[b].T.reshape(NKT, 128, S).transpose(1, 0, 2).astype(BFNP)))
        # per key tile bias: -shift, and -30 on masked-out keys
        kbias = -shift + (1.0 - am[b]) * -30.0
        kb_b.append(np.ascontiguousarray(
            kbias.reshape(NKT, 128).T.astype(np.float32)))

    in_maps = []
    for core in range(8):
        b, g = divmod(core, G)
        qs = slice(g * HPG * D, (g + 1) * HPG * D)
        ks = slice(g * D, (g + 1) * D)
        wo_own = Wo[qs, :]  # [512, HID]
        wo_pk = np.ascontiguousarray(
            wo_own.reshape(HPG, 128, NKT, 128).transpose(1, 0, 2, 3)
            .astype(BFNP))
        bo_pk = (np.asarray(bo, np.float32).reshape(NKT, 128).T
                 if g == 0 else np.zeros((128, NKT), np.float32))
        in_maps.append({
            "xt": xt_b[b],
            "wq": _pack_ft(Wq[:, qs], HPG * D),
            "wk": _pack_ft(Wk[:, ks], D),
            "wv": _pack_ft(Wv[:, ks], D),
            "wo": wo_pk,
            "bq": np.ascontiguousarray(
                np.asarray(bq, np.float32)[qs].reshape(HPG, D).T),
            "bk": np.asarray(bk, np.float32)[ks].reshape(D, 1).copy(),
            "bv": np.asarray(bv, np.float32)[ks].reshape(D, 1).copy(),
            "bo": np.ascontiguousarray(bo_pk),
            "kb": kb_b[b],
            "masks": masks.copy(),
            "ident": ident.copy(),
            "ones": ones.copy(),
        })

    global _last_in_maps
    _last_in_maps = in_maps
    res = run_bass_kernel_spmd(nc, in_maps, core_ids=list(range(8)))
    out = np.empty((B, S, HID), dtype=np.float32)
    for b in range(B):
        acc = res.results[4 * b]["out"].astype(np.float32)
        for g in range(1, G):
            acc += res.results[4 * b + g]["out"].astype(np.float32)
        out[b] = acc.T
    return out


# revision 12
# speedup vs baseline: 1.1056x; 1.1056x over previous
"""Causal GQA self-attention on 8 Trainium2 NeuronCores.

Problem: B=2, S=2048, HIDDEN=2048, 16 q-heads, 4 kv-heads, head_dim=128, fp32.

Sharding: core c = 4*b + g  (b = batch, g = head-group).  Each core owns batch
b and q-heads [4g, 4g+4) plus their shared kv-head g.  No collectives: each
core computes the partial O-projection  Wo[own rows].T @ attn_own  ->
[2048 hid, 2048 q] (bf16) and the host sums the 4 partials per batch.

Per-core pipeline (feature-major [d, s] layouts, all matmuls bf16):
  A. Projections per 512-col chunk: QT/KT/VT = W.T @ X.T accumulated over the
     16 128-row hid tiles; V obtained by PE-transposing VT (bias folded in).
  B. Attention per chunk/head: for each key-tile pair, S = KT_j.T @ QT (PSUM
     pair tile), P = exp(S/sqrt(d)) (one ACT op per pair), causal masks
     multiplied on the 4 diagonal tiles (DVE), then PV and the row-sum l
     accumulate on PE.  attnT = PV * (1/l broadcast).
  C. O-projection at the end with weight reuse over chunks:
     partial[128 hid-tile, q] += Wo_tile.T @ attnT, evac + DMA out per chunk.

P_FP8 variant: P is written fp8e4m3 (exp shifted by -C so max < 240) and the
row-sum uses a DoubleRow fp8 matmul over key-tile pairs (half the PE cycles of
the bf16 row-sum); PV keeps the bf16 V stationary with the fp8 moving P.
"""

import numpy as np
import ml_dtypes

HID = 2048
S = 2048
B = 2
NH = 16          # q heads total
D = 128          # head dim
G = 4            # head groups == cores per batch
HPG = NH // G    # q heads per group (4)
CH = 512         # seq chunk
NCH = S // CH    # 4 chunks
NKT = S // 128   # 16 key tiles / hid tiles
SCALE = 1.0 / float(np.sqrt(D))

P_FP8 = True     # P in fp8e4m3 + DoubleRow row-sum (else all-bf16)
EXP_C = 2.0      # global shift inside exp when P_FP8 (cancels in softmax)

F8NP = ml_dtypes.float8_e4m3
BFNP = ml_dtypes.bfloat16

_CACHED_NC = None
_CACHED_KEY = None
_last_in_maps = None


def _build_nc(mask_has_zeros=False):
    import concourse.mybir as mybir
    import concourse.tile as tile
    from concourse import bacc

    F32 = mybir.dt.float32
    BF = mybir.dt.bfloat16
    F8 = mybir.dt.float8e4
    PDT = F8 if P_FP8 else BF
    Copy = mybir.ActivationFunctionType.Copy
    Exp = mybir.ActivationFunctionType.Exp
    Ident = mybir.ActivationFunctionType.Identity
    DR = mybir.MatmulPerfMode.DoubleRow

    nc = bacc.Bacc("TRN2", target_bir_lowering=False, debug=False,
                   num_devices=8)

    xt_d = nc.declare_dram_parameter("xt", [128, NKT, S], BF, isOutput=False)
    wq_d = nc.declare_dram_parameter("wq", [128, NKT, HPG * D], BF, isOutput=False)
    wk_d = nc.declare_dram_parameter("wk", [128, NKT, D], BF, isOutput=False)
    wv_d = nc.declare_dram_parameter("wv", [128, NKT, D], BF, isOutput=False)
    wo_d = nc.declare_dram_parameter("wo", [128, HPG, NKT, D], BF, isOutput=False)
    bq_d = nc.declare_dram_parameter("bq", [D, HPG], F32, isOutput=False)
    bk_d = nc.declare_dram_parameter("bk", [D, 1], F32, isOutput=False)
    bv_d = nc.declare_dram_parameter("bv", [D, 1], F32, isOutput=False)
    bo_d = nc.declare_dram_parameter("bo", [D, NKT], F32, isOutput=False)
    kb_d = nc.declare_dram_parameter("kb", [128, NKT], F32, isOutput=False)
    mask_d = nc.declare_dram_parameter("masks", [128, HPG, CH], PDT, isOutput=False)
    id_d = nc.declare_dram_parameter("ident", [128, 128], BF, isOutput=False)
    ones_d = nc.declare_dram_parameter("ones", [128, 2, 16], PDT, isOutput=False)
    out_d = nc.declare_dram_parameter("out", [HID, S], BF, isOutput=True)

    with tile.TileContext(nc) as tc:
        with tc.tile_pool(name="persist", bufs=1) as persist:
            xt = persist.tile([128, NKT, S], BF)
            wq = persist.tile([128, NKT, HPG * D], BF)
            wk = persist.tile([128, NKT, D], BF)
            wv = persist.tile([128, NKT, D], BF)
            wo = persist.tile([128, HPG, NKT, D], BF)
            qt = persist.tile([128, HPG, S], BF)
            kt = persist.tile([128, S], BF)
            v16 = persist.tile([128, NKT, D], BF)
            at16 = persist.tile([128, HPG, S], BF)
            masks = persist.tile([128, HPG, CH], PDT)
            kb = persist.tile([128, NKT], F32)
            bq_sb = persist.tile([D, HPG], F32)
            bk_sb = persist.tile([D, 1], F32)
            bv_sb = persist.tile([D, 1], F32)
            bo_sb = persist.tile([D, NKT], F32)
            ident = persist.tile([128, 128], BF)
            ones_sb = persist.tile([128, 2, 16], PDT)

            # Phase A consumes (wq[t], wk[t], wv[t], xt[t, chunk]) in t-order
            # per chunk.  Weights ride the scalar HWDGE queue, xt the sync
            # HWDGE queue (chunk 0 first), small consts the gpsimd queue, so
            # the streams run in parallel and the PE never outruns the DMA.
            for t in range(NKT):
                nc.gpsimd.dma_start(out=wq[:, t, :], in_=wq_d.ap()[:, t, :])
                nc.sync.dma_start(out=wk[:, t, :], in_=wk_d.ap()[:, t, :])
                nc.sync.dma_start(out=wv[:, t, :], in_=wv_d.ap()[:, t, :])
                nc.gpsimd.dma_start(out=xt[:, t, 0:CH],
                                    in_=xt_d.ap()[:, t, 0:CH])
            nc.sync.dma_start(out=bq_sb, in_=bq_d.ap())
            nc.sync.dma_start(out=bk_sb, in_=bk_d.ap())
            nc.sync.dma_start(out=bv_sb, in_=bv_d.ap())
            nc.sync.dma_start(out=bo_sb, in_=bo_d.ap())
            nc.sync.dma_start(out=kb, in_=kb_d.ap())
            nc.sync.dma_start(out=ident, in_=id_d.ap())
            nc.sync.dma_start(out=ones_sb, in_=ones_d.ap())
            nc.sync.dma_start(out=masks, in_=mask_d.ap())
            for c in range(1, NCH):
                sq = slice(c * CH, (c + 1) * CH)
                for t in range(NKT):
                    nc.gpsimd.dma_start(out=xt[:, t, sq],
                                        in_=xt_d.ap()[:, t, sq])
            nc.sync.dma_start(out=wo, in_=wo_d.ap())

            # ================= Phase A: projections =================
            with (
                tc.tile_pool(name="psA", bufs=1, space="PSUM") as psA,
                tc.tile_pool(name="vts", bufs=2) as vts,
            ):
                for c in range(NCH):
                    sq = slice(c * CH, (c + 1) * CH)
                    ps_q = psA.tile([128, HPG, CH], F32, name="ps_q")
                    ps_k = psA.tile([128, CH], F32, name="ps_k")
                    ps_vt = psA.tile([128, CH], F32, name="ps_vt")
                    for t in range(NKT):
                        st, sp = (t == 0), (t == NKT - 1)
                        for h in range(HPG):
                            nc.tensor.matmul(
                                ps_q[:, h, :],
                                lhsT=wq[:, t, h * D:(h + 1) * D],
                                rhs=xt[:, t, sq], start=st, stop=sp)
                        nc.tensor.matmul(ps_k, lhsT=wk[:, t, :],
                                         rhs=xt[:, t, sq], start=st, stop=sp)
                        nc.tensor.matmul(ps_vt, lhsT=wv[:, t, :],
                                         rhs=xt[:, t, sq], start=st, stop=sp)
                    for h in range(HPG):
                        nc.vector.tensor_scalar_add(qt[:, h, sq],
                                                    ps_q[:, h, :],
                                                    bq_sb[:, h:h + 1])
                    nc.vector.tensor_scalar_add(kt[:, sq], ps_k, bk_sb)
                    vt_t = vts.tile([128, CH], BF, name="vt_t")
                    nc.scalar.activation(vt_t, ps_vt, Ident, bias=bv_sb[:, 0:1])
                    for u in range(4):
                        ps_tp = psA.tile([128, 128], BF, name="ps_tp",
                                         bufs=2)
                        nc.tensor.transpose(
                            ps_tp, vt_t[:, u * 128:(u + 1) * 128], ident)
                        nc.scalar.activation(v16[:, 4 * c + u, :], ps_tp, Copy)

            # ================= Phase B: attention =================
            with (
                tc.tile_pool(name="psS", bufs=2, space="PSUM") as psS,
                tc.tile_pool(name="psPV", bufs=3, space="PSUM") as psPV,
                tc.tile_pool(name="psL", bufs=1, space="PSUM") as psL,
                tc.tile_pool(name="pp", bufs=3) as ppool,
                tc.tile_pool(name="rbp", bufs=2) as rbp,
            ):
                for c in range(NCH):
                    sq = slice(c * CH, (c + 1) * CH)
                    njt = 4 * c + 4
                    for h in range(HPG):
                        ps_pv = psPV.tile([128, CH], F32, name="ps_pv")
                        ps_l = psL.tile([16, CH], F32, name="ps_l")
                        for pi in range(njt // 2):
                            j0 = 2 * pi
                            last = (pi == njt // 2 - 1)
                            # the last pair (key tiles 4c+2, 4c+3) only
                            # reaches queries >= 256 within the chunk
                            off = 256 if last else 0
                            w = CH - off
                            cs = slice(off, CH)
                            qcs = slice(c * CH + off, (c + 1) * CH)
                            ps_s = psS.tile([128, 2, CH], F32, name="ps_s")
                            for sl in range(2):
                                j = j0 + sl
                                nc.tensor.matmul(
                                    ps_s[:, sl, cs],
                                    lhsT=kt[:, j * 128:(j + 1) * 128],
                                    rhs=qt[:, h, qcs], start=True, stop=True)
                            p16 = ppool.tile([128, 2, CH], PDT, name="p16")
                            if mask_has_zeros:
                                for sl in range(2):
                                    j = j0 + sl
                                    nc.scalar.activation(
                                        p16[:, sl, cs], ps_s[:, sl, cs], Exp,
                                        scale=SCALE, bias=kb[:, j:j + 1])
                            else:
                                # kb is constant -shift when the mask has no
                                # zeros, so one bias AP serves the whole pair
                                nc.scalar.activation(
                                    p16[:, :, cs], ps_s[:, :, cs], Exp,
                                    scale=SCALE, bias=kb[:, 0:1])
                            if pi == 2 * c:  # diagonal pair 0: r = 0, 1
                                nc.vector.tensor_mul(
                                    p16[:, 0, 0:128], p16[:, 0, 0:128],
                                    masks[:, 0, 0:128])
                                nc.vector.tensor_mul(
                                    p16[:, 1, 0:256], p16[:, 1, 0:256],
                                    masks[:, 1, 0:256])
                            if last:         # diagonal pair 1: r = 2, 3
                                nc.vector.tensor_mul(
                                    p16[:, 0, 256:384], p16[:, 0, 256:384],
                                    masks[:, 2, 256:384])
                                nc.vector.tensor_mul(
                                    p16[:, 1, 256:512], p16[:, 1, 256:512],
                                    masks[:, 3, 256:512])
                            for sl in range(2):
                                j = j0 + sl
                                nc.tensor.matmul(
                                    ps_pv[:, cs], lhsT=v16[:, j, :],
                                    rhs=p16[:, sl, cs],
                                    start=(j == 0), stop=(j == njt - 1))
                            if P_FP8:
                                nc.tensor.matmul(
                                    ps_l[:, cs], lhsT=ones_sb,
                                    rhs=p16[:, :, cs], perf_mode=DR,
                                    start=(pi == 0), stop=last)
                            else:
                                for sl in range(2):
                                    j = j0 + sl
                                    nc.tensor.matmul(
                                        ps_l[:, cs], lhsT=ones_sb[:, 0, :],
                                        rhs=p16[:, sl, cs],
                                        start=(j == 0), stop=(j == njt - 1))
                        rl = rbp.tile([1, CH], F32, name="rl")
                        nc.vector.reciprocal_approx_fast(rl, ps_l[0:1, :])
                        rb = rbp.tile([128, CH], F32, name="rb")
                        nc.gpsimd.partition_broadcast(rb, rl, channels=128)
                        nc.vector.tensor_mul(at16[:, h, sq], ps_pv, rb)

            # ================= Phase C: partial O-projection =================
            with (
                tc.tile_pool(name="psO", bufs=2, space="PSUM") as psO,
                tc.tile_pool(name="ost", bufs=6) as ost,
            ):
                for tcol in range(NKT):
                    ps_o = psO.tile([128, NCH, CH], F32, name="ps_o")
                    for ft in range(HPG):
                        for c in range(NCH):
                            nc.tensor.matmul(
                                ps_o[:, c, :], lhsT=wo[:, ft, tcol, :],
                                rhs=at16[:, ft, c * CH:(c + 1) * CH],
                                start=(ft == 0), stop=(ft == HPG - 1))
                    for c in range(NCH):
                        o_sb = ost.tile([128, CH], BF, name="o_sb")
                        nc.vector.tensor_scalar_add(o_sb, ps_o[:, c, :],
                                                    bo_sb[:, tcol:tcol + 1])
                        eng = nc.sync if (c % 2 == 0) else nc.scalar
                        eng.dma_start(
                            out=out_d[tcol * 128:(tcol + 1) * 128,
                                      c * CH:(c + 1) * CH],
                            in_=o_sb)

    nc.compile()
    return nc


def _host_consts():
    # tri masks for the diagonal key tiles: masks[p, r, col] = col >= 128r + p
    col = np.arange(CH)[None, None, :]
    r = np.arange(HPG)[None, :, None]
    p = np.arange(128)[:, None, None]
    pdt = F8NP if P_FP8 else BFNP
    masks = (col >= 128 * r + p).astype(pdt)
    ident = np.eye(128, dtype=BFNP)
    ones = np.ones((128, 2, 16), dtype=pdt)
    return masks, ident, ones


def _pack_ft(w, m):
    # [HID, m] -> [128, NKT, m]: row 128*t + p goes to [p, t, :]
    return np.ascontiguousarray(
        w.reshape(NKT, 128, m).transpose(1, 0, 2).astype(BFNP))


def kernel(hidden_states, attention_mask, Wq, bq, Wk, bk, Wv, bv, Wo, bo):
    from concourse.bass_utils import run_bass_kernel_spmd

    X = np.asarray(hidden_states, dtype=np.float32)
    am = np.asarray(attention_mask).astype(np.float32)  # [B, S] key mask
    Wq = np.asarray(Wq, np.float32)
    Wk = np.asarray(Wk, np.float32)
    Wv = np.asarray(Wv, np.float32)
    Wo = np.asarray(Wo, np.float32)
    mask_has_zeros = bool((am == 0.0).any())

    global _CACHED_NC, _CACHED_KEY
    if _CACHED_NC is None or _CACHED_KEY != mask_has_zeros:
        _CACHED_NC = _build_nc(mask_has_zeros)
        _CACHED_KEY = mask_has_zeros
    nc = _CACHED_NC

    masks, ident, ones = _host_consts()
    shift = EXP_C if P_FP8 else 0.0

    xt_b = []
    kb_b = []
    for b in range(B):
        xt_b.append(np.ascontiguousarray(
            X[b].T.reshape(NKT, 128, S).transpose(1, 0, 2).astype(BFNP)))
        # per key tile bias: -shift, and -30 on masked-out keys
        kbias = -shift + (1.0 - am[b]) * -30.0
        kb_b.append(np.ascontiguousarray(
            kbias.reshape(NKT, 128).T.astype(np.float32)))

    in_maps = []
    for core in range(8):
        b, g = divmod(core, G)
        qs = slice(g * HPG * D, (g + 1) * HPG * D)
        ks = slice(g * D, (g + 1) * D)
        wo_own = Wo[qs, :]  # [512, HID]
        wo_pk = np.ascontiguousarray(
            wo_own.reshape(HPG, 128, NKT, 128).transpose(1, 0, 2, 3)
            .astype(BFNP))
        bo_pk = (np.asarray(bo, np.float32).reshape(NKT, 128).T
                 if g == 0 else np.zeros((128, NKT), np.float32))
        in_maps.append({
            "xt": xt_b[b],
            "wq": _pack_ft(Wq[:, qs], HPG * D),
            "wk": _pack_ft(Wk[:, ks], D),
            "wv": _pack_ft(Wv[:, ks], D),
            "wo": wo_pk,
            "bq": np.ascontiguousarray(
                np.asarray(bq, np.float32)[qs].reshape(HPG, D).T),
            "bk": np.asarray(bk, np.float32)[ks].reshape(D, 1).copy(),
            "bv": np.asarray(bv, np.float32)[ks].reshape(D, 1).copy(),
            "bo": np.ascontiguousarray(bo_pk),
            "kb": kb_b[b],
            "masks": masks.copy(),
            "ident": ident.copy(),
            "ones": ones.copy(),
        })

    global _last_in_maps
    _last_in_maps = in_maps
    res = run_bass_kernel_spmd(nc, in_maps, core_ids=list(range(8)))
    out = np.empty((B, S, HID), dtype=np.float32)
    for b in range(B):
        acc = res.results[4 * b]["out"].astype(np.float32)
        for g in range(1, G):
            acc += res.results[4 * b + g]["out"].astype(np.float32)
        out[b] = acc.T
    return out


# revision 14
# speedup vs baseline: 1.1110x; 1.0049x over previous
"""Causal GQA self-attention on 8 Trainium2 NeuronCores.

Problem: B=2, S=2048, HIDDEN=2048, 16 q-heads, 4 kv-heads, head_dim=128, fp32.

Sharding: core c = 4*b + g  (b = batch, g = head-group).  Each core owns batch
b and q-heads [4g, 4g+4) plus their shared kv-head g.  No collectives: each
core computes the partial O-projection  Wo[own rows].T @ attn_own  ->
[2048 hid, 2048 q] (bf16) and the host sums the 4 partials per batch.

Per-core pipeline (feature-major [d, s] layouts, all matmuls bf16):
  A. Projections per 512-col chunk: QT/KT/VT = W.T @ X.T accumulated over the
     16 128-row hid tiles; V obtained by PE-transposing VT (bias folded in).
  B. Attention per chunk/head: for each key-tile pair, S = KT_j.T @ QT (PSUM
     pair tile), P = exp(S/sqrt(d)) (one ACT op per pair), causal masks
     multiplied on the 4 diagonal tiles (DVE), then PV and the row-sum l
     accumulate on PE.  attnT = PV * (1/l broadcast).
  C. O-projection at the end with weight reuse over chunks:
     partial[128 hid-tile, q] += Wo_tile.T @ attnT, evac + DMA out per chunk.

P_FP8 variant: P is written fp8e4m3 (exp shifted by -C so max < 240) and the
row-sum uses a DoubleRow fp8 matmul over key-tile pairs (half the PE cycles of
the bf16 row-sum); PV keeps the bf16 V stationary with the fp8 moving P.
"""

import numpy as np
import ml_dtypes

HID = 2048
S = 2048
B = 2
NH = 16          # q heads total
D = 128          # head dim
G = 4            # head groups == cores per batch
HPG = NH // G    # q heads per group (4)
CH = 512         # seq chunk
NCH = S // CH    # 4 chunks
NKT = S // 128   # 16 key tiles / hid tiles
SCALE = 1.0 / float(np.sqrt(D))

P_FP8 = True     # P in fp8e4m3 + DoubleRow row-sum (else all-bf16)
EXP_C = 2.0      # global shift inside exp when P_FP8 (cancels in softmax)

F8NP = ml_dtypes.float8_e4m3
BFNP = ml_dtypes.bfloat16

_CACHED_NC = None
_CACHED_KEY = None
_last_in_maps = None


def _build_nc(mask_has_zeros=False):
    import concourse.mybir as mybir
    import concourse.tile as tile
    from concourse import bacc

    F32 = mybir.dt.float32
    BF = mybir.dt.bfloat16
    F8 = mybir.dt.float8e4
    PDT = F8 if P_FP8 else BF
    Copy = mybir.ActivationFunctionType.Copy
    Exp = mybir.ActivationFunctionType.Exp
    Ident = mybir.ActivationFunctionType.Identity
    DR = mybir.MatmulPerfMode.DoubleRow

    nc = bacc.Bacc("TRN2", target_bir_lowering=False, debug=False,
                   num_devices=8)

    xt_d = nc.declare_dram_parameter("xt", [128, NKT, S], BF, isOutput=False)
    wq_d = nc.declare_dram_parameter("wq", [128, NKT, HPG * D], BF, isOutput=False)
    wk_d = nc.declare_dram_parameter("wk", [128, NKT, D], BF, isOutput=False)
    wv_d = nc.declare_dram_parameter("wv", [128, NKT, D], BF, isOutput=False)
    wo_d = nc.declare_dram_parameter("wo", [128, HPG, NKT, D], BF, isOutput=False)
    bq_d = nc.declare_dram_parameter("bq", [D, HPG], F32, isOutput=False)
    bk_d = nc.declare_dram_parameter("bk", [D, 1], F32, isOutput=False)
    bv_d = nc.declare_dram_parameter("bv", [D, 1], F32, isOutput=False)
    bo_d = nc.declare_dram_parameter("bo", [D, NKT], F32, isOutput=False)
    kb_d = nc.declare_dram_parameter("kb", [128, NKT], F32, isOutput=False)
    mask_d = nc.declare_dram_parameter("masks", [128, HPG, CH], PDT, isOutput=False)
    id_d = nc.declare_dram_parameter("ident", [128, 128], BF, isOutput=False)
    ones_d = nc.declare_dram_parameter("ones", [128, 2, 16], PDT, isOutput=False)
    out_d = nc.declare_dram_parameter("out", [HID, S], BF, isOutput=True)

    with tile.TileContext(nc) as tc:
        with tc.tile_pool(name="persist", bufs=1) as persist:
            xt = persist.tile([128, NKT, S], BF)
            wq = persist.tile([128, NKT, HPG * D], BF)
            wk = persist.tile([128, NKT, D], BF)
            wv = persist.tile([128, NKT, D], BF)
            wo = persist.tile([128, HPG, NKT, D], BF)
            qt = persist.tile([128, HPG, S], BF)
            kt = persist.tile([128, S], BF)
            v16 = persist.tile([128, NKT, D], BF)
            at16 = persist.tile([128, HPG, S], BF)
            masks = persist.tile([128, HPG, CH], PDT)
            kb = persist.tile([128, NKT], F32)
            bq_sb = persist.tile([D, HPG], F32)
            bk_sb = persist.tile([D, 1], F32)
            bv_sb = persist.tile([D, 1], F32)
            bo_sb = persist.tile([D, NKT], F32)
            ident = persist.tile([128, 128], BF)
            ones_sb = persist.tile([128, 2, 16], PDT)

            # Phase A consumes (wq[t], wk[t], wv[t], xt[t, chunk]) in t-order
            # per chunk.  Weights ride the scalar HWDGE queue, xt the sync
            # HWDGE queue (chunk 0 first), small consts the gpsimd queue, so
            # the streams run in parallel and the PE never outruns the DMA.
            for t in range(NKT):
                nc.gpsimd.dma_start(out=wq[:, t, :], in_=wq_d.ap()[:, t, :])
                nc.sync.dma_start(out=wk[:, t, :], in_=wk_d.ap()[:, t, :])
                nc.sync.dma_start(out=wv[:, t, :], in_=wv_d.ap()[:, t, :])
                nc.gpsimd.dma_start(out=xt[:, t, 0:CH],
                                    in_=xt_d.ap()[:, t, 0:CH])
            nc.sync.dma_start(out=bq_sb, in_=bq_d.ap())
            nc.sync.dma_start(out=bk_sb, in_=bk_d.ap())
            nc.sync.dma_start(out=bv_sb, in_=bv_d.ap())
            nc.sync.dma_start(out=bo_sb, in_=bo_d.ap())
            nc.sync.dma_start(out=kb, in_=kb_d.ap())
            nc.sync.dma_start(out=ident, in_=id_d.ap())
            nc.sync.dma_start(out=ones_sb, in_=ones_d.ap())
            nc.sync.dma_start(out=masks, in_=mask_d.ap())
            for c in range(1, NCH):
                sq = slice(c * CH, (c + 1) * CH)
                for t in range(NKT):
                    nc.gpsimd.dma_start(out=xt[:, t, sq],
                                        in_=xt_d.ap()[:, t, sq])
            nc.sync.dma_start(out=wo, in_=wo_d.ap())

            # ================= Phase A: projections =================
            with (
                tc.tile_pool(name="psA", bufs=1, space="PSUM") as psA,
                tc.tile_pool(name="vts", bufs=2) as vts,
            ):
                for c in range(NCH):
                    sq = slice(c * CH, (c + 1) * CH)
                    ps_q = psA.tile([128, HPG, CH], F32, name="ps_q")
                    ps_k = psA.tile([128, CH], F32, name="ps_k")
                    ps_vt = psA.tile([128, CH], F32, name="ps_vt")
                    for t in range(NKT):
                        st, sp = (t == 0), (t == NKT - 1)
                        for h in range(HPG):
                            nc.tensor.matmul(
                                ps_q[:, h, :],
                                lhsT=wq[:, t, h * D:(h + 1) * D],
                                rhs=xt[:, t, sq], start=st, stop=sp)
                        nc.tensor.matmul(ps_k, lhsT=wk[:, t, :],
                                         rhs=xt[:, t, sq], start=st, stop=sp)
                        nc.tensor.matmul(ps_vt, lhsT=wv[:, t, :],
                                         rhs=xt[:, t, sq], start=st, stop=sp)
                    # split the four Q evacs between DVE and ACT so the
                    # ps_q banks free up before the next chunk's matmuls
                    for h in range(HPG):
                        if h < 2:
                            nc.vector.tensor_scalar_add(qt[:, h, sq],
                                                        ps_q[:, h, :],
                                                        bq_sb[:, h:h + 1])
                        else:
                            nc.scalar.activation(qt[:, h, sq], ps_q[:, h, :],
                                                 Ident, bias=bq_sb[:, h:h + 1])
                    nc.vector.tensor_scalar_add(kt[:, sq], ps_k, bk_sb)
                    vt_t = vts.tile([128, CH], BF, name="vt_t")
                    nc.scalar.activation(vt_t, ps_vt, Ident, bias=bv_sb[:, 0:1])
                    for u in range(4):
                        ps_tp = psA.tile([128, 128], BF, name="ps_tp",
                                         bufs=2)
                        nc.tensor.transpose(
                            ps_tp, vt_t[:, u * 128:(u + 1) * 128], ident)
                        nc.scalar.activation(v16[:, 4 * c + u, :], ps_tp, Copy)

            # ================= Phase B: attention =================
            with (
                tc.tile_pool(name="psS", bufs=2, space="PSUM") as psS,
                tc.tile_pool(name="psPV", bufs=2, space="PSUM") as psPV,
                tc.tile_pool(name="psL", bufs=2, space="PSUM") as psL,
                tc.tile_pool(name="pp", bufs=4) as ppool,
                tc.tile_pool(name="rbp", bufs=2) as rbp,
            ):
                # interleave the two chunks of a pair at head granularity so
                # one head's normalize chain hides under the other chunk's
                # matmul stream (matters most for the short early chunks)
                order = [(c, h) for c0 in (0, 2) for h in range(HPG)
                         for c in (c0, c0 + 1)]
                for c, h in order:
                    sq = slice(c * CH, (c + 1) * CH)
                    njt = 4 * c + 4
                    if True:
                        ps_pv = psPV.tile([128, CH], F32, name="ps_pv")
                        ps_l = psL.tile([16, CH], F32, name="ps_l")
                        for pi in range(njt // 2):
                            j0 = 2 * pi
                            last = (pi == njt // 2 - 1)
                            # the last pair (key tiles 4c+2, 4c+3) only
                            # reaches queries >= 256 within the chunk
                            off = 256 if last else 0
                            w = CH - off
                            cs = slice(off, CH)
                            qcs = slice(c * CH + off, (c + 1) * CH)
                            ps_s = psS.tile([128, 2, CH], F32, name="ps_s")
                            for sl in range(2):
                                j = j0 + sl
                                nc.tensor.matmul(
                                    ps_s[:, sl, cs],
                                    lhsT=kt[:, j * 128:(j + 1) * 128],
                                    rhs=qt[:, h, qcs], start=True, stop=True)
                            p16 = ppool.tile([128, 2, CH], PDT, name="p16")
                            if mask_has_zeros:
                                for sl in range(2):
                                    j = j0 + sl
                                    nc.scalar.activation(
                                        p16[:, sl, cs], ps_s[:, sl, cs], Exp,
                                        scale=SCALE, bias=kb[:, j:j + 1])
                            else:
                                # kb is constant -shift when the mask has no
                                # zeros, so one bias AP serves the whole pair
                                nc.scalar.activation(
                                    p16[:, :, cs], ps_s[:, :, cs], Exp,
                                    scale=SCALE, bias=kb[:, 0:1])
                            if pi == 2 * c:  # diagonal pair 0: r = 0, 1
                                nc.vector.tensor_mul(
                                    p16[:, 0, 0:128], p16[:, 0, 0:128],
                                    masks[:, 0, 0:128])
                                nc.vector.tensor_mul(
                                    p16[:, 1, 0:256], p16[:, 1, 0:256],
                                    masks[:, 1, 0:256])
                            if last:         # diagonal pair 1: r = 2, 3
                                nc.vector.tensor_mul(
                                    p16[:, 0, 256:384], p16[:, 0, 256:384],
                                    masks[:, 2, 256:384])
                                nc.vector.tensor_mul(
                                    p16[:, 1, 256:512], p16[:, 1, 256:512],
                                    masks[:, 3, 256:512])
                            for sl in range(2):
                                j = j0 + sl
                                nc.tensor.matmul(
                                    ps_pv[:, cs], lhsT=v16[:, j, :],
                                    rhs=p16[:, sl, cs],
                                    start=(j == 0), stop=(j == njt - 1))
                            if P_FP8:
                                nc.tensor.matmul(
                                    ps_l[:, cs], lhsT=ones_sb,
                                    rhs=p16[:, :, cs], perf_mode=DR,
                                    start=(pi == 0), stop=last)
                            else:
                                for sl in range(2):
                                    j = j0 + sl
                                    nc.tensor.matmul(
                                        ps_l[:, cs], lhsT=ones_sb[:, 0, :],
                                        rhs=p16[:, sl, cs],
                                        start=(j == 0), stop=(j == njt - 1))
                        rl = rbp.tile([1, CH], F32, name="rl")
                        nc.vector.reciprocal_approx_fast(rl, ps_l[0:1, :])
                        rb = rbp.tile([128, CH], F32, name="rb")
                        nc.gpsimd.partition_broadcast(rb, rl, channels=128)
                        nc.vector.tensor_mul(at16[:, h, sq], ps_pv, rb)

            # ================= Phase C: partial O-projection =================
            with (
                tc.tile_pool(name="psO", bufs=2, space="PSUM") as psO,
                tc.tile_pool(name="ost", bufs=6) as ost,
            ):
                for tcol in range(NKT):
                    ps_o = psO.tile([128, NCH, CH], F32, name="ps_o")
                    for ft in range(HPG):
                        for c in range(NCH):
                            nc.tensor.matmul(
                                ps_o[:, c, :], lhsT=wo[:, ft, tcol, :],
                                rhs=at16[:, ft, c * CH:(c + 1) * CH],
                                start=(ft == 0), stop=(ft == HPG - 1))
                    for c in range(NCH):
                        o_sb = ost.tile([128, CH], BF, name="o_sb")
                        nc.vector.tensor_scalar_add(o_sb, ps_o[:, c, :],
                                                    bo_sb[:, tcol:tcol + 1])
                        eng = nc.sync if (c % 2 == 0) else nc.scalar
                        eng.dma_start(
                            out=out_d[tcol * 128:(tcol + 1) * 128,
                                      c * CH:(c + 1) * CH],
                            in_=o_sb)

    nc.compile()
    return nc


def _host_consts():
    # tri masks for the diagonal key tiles: masks[p, r, col] = col >= 128r + p
    col = np.arange(CH)[None, None, :]
    r = np.arange(HPG)[None, :, None]
    p = np.arange(128)[:, None, None]
    pdt = F8NP if P_FP8 else BFNP
    masks = (col >= 128 * r + p).astype(pdt)
    ident = np.eye(128, dtype=BFNP)
    ones = np.ones((128, 2, 16), dtype=pdt)
    return masks, ident, ones


def _pack_ft(w, m):
    # [HID, m] -> [128, NKT, m]: row 128*t + p goes to [p, t, :]
    return np.ascontiguousarray(
        w.reshape(NKT, 128, m).transpose(1, 0, 2).astype(BFNP))


def kernel(hidden_states, attention_mask, Wq, bq, Wk, bk, Wv, bv, Wo, bo):
    from concourse.bass_utils import run_bass_kernel_spmd

    X = np.asarray(hidden_states, dtype=np.float32)
    am = np.asarray(attention_mask).astype(np.float32)  # [B, S] key mask
    Wq = np.asarray(Wq, np.float32)
    Wk = np.asarray(Wk, np.float32)
    Wv = np.asarray(Wv, np.float32)
    Wo = np.asarray(Wo, np.float32)
    mask_has_zeros = bool((am == 0.0).any())

    global _CACHED_NC, _CACHED_KEY
    if _CACHED_NC is None or _CACHED_KEY != mask_has_zeros:
        _CACHED_NC = _build_nc(mask_has_zeros)
        _CACHED_KEY = mask_has_zeros
    nc = _CACHED_NC

    masks, ident, ones = _host_consts()
    shift = EXP_C if P_FP8 else 0.0

    xt_b = []
    kb_b = []
    for b in range(B):
        xt_b.append(np.ascontiguousarray(
            X[b].T.reshape(NKT, 128, S).transpose(1, 0, 2).astype(BFNP)))
        # per key tile bias: -shift, and -30 on masked-out keys
        kbias = -shift + (1.0 - am[b]) * -30.0
        kb_b.append(np.ascontiguousarray(
            kbias.reshape(NKT, 128).T.astype(np.float32)))

    in_maps = []
    for core in range(8):
        b, g = divmod(core, G)
        qs = slice(g * HPG * D, (g + 1) * HPG * D)
        ks = slice(g * D, (g + 1) * D)
        wo_own = Wo[qs, :]  # [512, HID]
        wo_pk = np.ascontiguousarray(
            wo_own.reshape(HPG, 128, NKT, 128).transpose(1, 0, 2, 3)
            .astype(BFNP))
        bo_pk = (np.asarray(bo, np.float32).reshape(NKT, 128).T
                 if g == 0 else np.zeros((128, NKT), np.float32))
        in_maps.append({
            "xt": xt_b[b],
            "wq": _pack_ft(Wq[:, qs], HPG * D),
            "wk": _pack_ft(Wk[:, ks], D),
            "wv": _pack_ft(Wv[:, ks], D),
            "wo": wo_pk,
            "bq": np.ascontiguousarray(
                np.asarray(bq, np.float32)[qs].reshape(HPG, D).T),
            "bk": np.asarray(bk, np.float32)[ks].reshape(D, 1).copy(),
            "bv": np.asarray(bv, np.float32)[ks].reshape(D, 1).copy(),
            "bo": np.ascontiguousarray(bo_pk),
            "kb": kb_b[b],
            "masks": masks.copy(),
            "ident": ident.copy(),
            "ones": ones.copy(),
        })

    global _last_in_maps
    _last_in_maps = in_maps
    res = run_bass_kernel_spmd(nc, in_maps, core_ids=list(range(8)))
    out = np.empty((B, S, HID), dtype=np.float32)
    for b in range(B):
        acc = res.results[4 * b]["out"].astype(np.float32)
        for g in range(1, G):
            acc += res.results[4 * b + g]["out"].astype(np.float32)
        out[b] = acc.T
    return out
